# revision 21
# baseline (speedup 1.0000x reference)
"""Trainium2 Bass kernel: causal GQA self-attention (B=1, S=2048, D=2048,
H=16 q-heads, 4 kv-heads, head_dim=128) with q/k RMS-norm, full-head RoPE,
per-head q gain, and output projection.

Sharding: tensor-parallel over 8 NeuronCores. Core i owns q-heads {2i, 2i+1}
and kv-head i//2: it computes its two heads' attention output and a partial
output projection (columns 256i..256i+256 of the y-concat contracted against
Wproj), returning a full-shape [2048, 2048] f32 partial that the host sums
across cores (the "all-reduce").

Everything runs in bf16 on the PE array with f32 PSUM accumulation; the
normalizations are applied in f32 (k's 1/rms rides the softmax-exp's
per-partition scale, q's is multiplied in f32 before rounding to bf16).
"""

import math
from contextlib import ExitStack

import numpy as np
import ml_dtypes

import concourse.bass as bass
import concourse.tile as tile
from concourse import mybir, library_config
from concourse.bass_isa import ReduceOp
from concourse.bass_utils import run_bass_kernel_spmd

BF16 = mybir.dt.bfloat16
F32 = mybir.dt.float32
NP_BF16 = ml_dtypes.bfloat16

S = 2048
D = 2048
H = 16
HKV = 4
HD = 128
NCORES = 8
QH_PER_CORE = H // NCORES          # 2
M_PER_CORE = QH_PER_CORE * HD      # 256
NT = S // 128                      # 16 s-tiles / d-blocks
BASE = 10000.0
EPS = float(np.finfo(np.float32).eps)

AF = mybir.ActivationFunctionType
ALU = mybir.AluOpType


_MAXW = 1  # max sync-wait commands this walrus accepts per instruction


def _install_drain_split_patch():
    """The walrus build here only accepts <=2 sync-wait commands per
    instruction.  Tile attaches one wait per producer semaphore, which can be
    many.  Split the excess onto same-engine NoOps committed immediately
    before the instruction (same program point, so ordering semantics are
    unchanged), and likewise chain the kernel-tail drain."""
    from concourse.vector_clock import ScopedClock
    import bass_rust

    if getattr(tile.TileContext, "_drain_split_patched", False):
        return

    orig_commit = tile.TileContext._commit_instruction

    def _commit_instruction(self, inst, lazy_reg_writes=True):
        si = getattr(inst, "sync_info", None)
        if (si is not None and si.on_wait and len(si.on_wait) > _MAXW
                and inst.engine != mybir.EngineType.Unassigned
                and isinstance(inst, mybir.Instruction)):
            waits = list(si.on_wait)
            excess, keep = waits[:-_MAXW], waits[-_MAXW:]
            for k in range(0, len(excess), _MAXW):
                nop = mybir.InstNoOp(
                    name=f"{inst.name}-wsplit{k}", ins=[], outs=[])
                nop.engine = inst.engine
                nop.sync_info = bass_rust.SyncInfo(
                    on_wait=excess[k:k + _MAXW], on_update=[])
                self._add_instruction(nop)
            si.on_wait = keep
            inst.sync_info = si
        return orig_commit(self, inst, lazy_reg_writes)

    tile.TileContext._commit_instruction = _commit_instruction

    def _drain_and_barrier(self, tick_clock, wait_clock):
        nc = self.nc
        drain_inst = nc.sync.drain()
        wait_clock.add_sem_waits(
            drain_inst.ins, ScopedClock({None: tick_clock.global_clock}))
        mi = drain_inst.ins
        si = mi.sync_info
        if si is not None and si.on_wait and len(si.on_wait) > 1:
            waits = list(si.on_wait)
            si.on_wait = waits[:1]
            mi.sync_info = si
            for w in waits[1:]:
                d2 = nc.sync.drain()
                s2 = d2.ins.sync_info
                if s2 is None:
                    s2 = type(si)(on_wait=[w], on_update=[])
                else:
                    s2.on_wait = [w]
                d2.ins.sync_info = s2
        nc.all_engine_barrier()
        assert self.sems is not None
        popped = nc._tile_sem_poison_stack.pop()
        assert popped is self._sem_poison
        nc.clear_and_free_semaphores(list(self.sems.allocated().values()))
        nc.all_engine_barrier()

    tile.TileContext._drain_and_barrier = _drain_and_barrier
    tile.TileContext._drain_split_patched = True


_install_drain_split_patch()


def _build_program():
    nc = bass.Bass()

    # ---- DRAM I/O ----
    xt_d = nc.declare_dram_parameter("xt", [D, S], BF16, isOutput=False)
    wqt_d = nc.declare_dram_parameter("wqt", [D, M_PER_CORE], BF16, isOutput=False)
    wkt_d = nc.declare_dram_parameter("wkt", [D, HD], BF16, isOutput=False)
    wvt_d = nc.declare_dram_parameter("wvt", [D, HD], BF16, isOutput=False)
    pt_d = nc.declare_dram_parameter("pt", [M_PER_CORE, D], BF16, isOutput=False)
    cost_d = nc.declare_dram_parameter("cost", [HD, S], BF16, isOutput=False)
    sint_d = nc.declare_dram_parameter("sint", [HD, S], BF16, isOutput=False)
    mask_d = nc.declare_dram_parameter("maskd", [128, 128], BF16, isOutput=False)
    ident_d = nc.declare_dram_parameter("ident", [128, 128], BF16, isOutput=False)
    qkc_d = nc.declare_dram_parameter("qkc", [128, 6], F32, isOutput=False)
    out_d = nc.declare_dram_parameter("partial", [S, D], F32, isOutput=True)

    with tile.TileContext(nc) as tc:
        with ExitStack() as ctx:
            pers = ctx.enter_context(tc.tile_pool(name="pers", bufs=1))
            tp_f32 = ctx.enter_context(tc.tile_pool(name="tp_f32", bufs=2))
            tp_half = ctx.enter_context(tc.tile_pool(name="tp_half", bufs=1))
            tp_q = ctx.enter_context(tc.tile_pool(name="tp_q", bufs=2))
            tp_probs = ctx.enter_context(tc.tile_pool(name="tp_probs", bufs=2))
            tp_row = ctx.enter_context(tc.tile_pool(name="tp_row", bufs=1))
            tp_y = ctx.enter_context(tc.tile_pool(name="tp_y", bufs=3))
            tp_rec = ctx.enter_context(tc.tile_pool(name="tp_rec", bufs=3))
            tp_out = ctx.enter_context(tc.tile_pool(name="tp_out", bufs=2))
            ps_big = ctx.enter_context(
                tc.tile_pool(name="ps_big", bufs=6, space="PSUM"))
            ps_sm = ctx.enter_context(
                tc.tile_pool(name="ps_sm", bufs=2, space="PSUM"))
            dram = ctx.enter_context(
                tc.tile_pool(name="dram", bufs=1, space="DRAM"))

            # ---- Phase A: load everything ----
            xt = []
            for t in range(NT):
                xti = pers.tile([128, S], BF16, tag=f"xt{t}", name=f"xt{t}")
                nc.sync.dma_start(xti[:], xt_d[128 * t:128 * (t + 1), :])
                xt.append(xti)
            wqt = []
            for t in range(NT):
                w = pers.tile([128, M_PER_CORE], BF16, tag=f"wqt{t}", name=f"wqt{t}")
                nc.sync.dma_start(w[:], wqt_d[128 * t:128 * (t + 1), :])
                wqt.append(w)
            wkt = []
            wvt = []
            for t in range(NT):
                w = pers.tile([128, HD], BF16, tag=f"wkt{t}", name=f"wkt{t}")
                nc.sync.dma_start(w[:], wkt_d[128 * t:128 * (t + 1), :])
                wkt.append(w)
                w = pers.tile([128, HD], BF16, tag=f"wvt{t}", name=f"wvt{t}")
                nc.sync.dma_start(w[:], wvt_d[128 * t:128 * (t + 1), :])
                wvt.append(w)
            pt = []
            for h in range(QH_PER_CORE):
                w = pers.tile([128, D], BF16, tag=f"pt{h}", name=f"pt{h}")
                nc.sync.dma_start(w[:], pt_d[128 * h:128 * (h + 1), :])
                pt.append(w)
            cost = pers.tile([HD, S], BF16, tag="cost", name="cost")
            nc.sync.dma_start(cost[:], cost_d[:])
            sint = pers.tile([HD, S], BF16, tag="sint", name="sint")
            nc.sync.dma_start(sint[:], sint_d[:])
            maskd = pers.tile([128, 128], BF16, tag="maskd", name="maskd")
            nc.sync.dma_start(maskd[:], mask_d[:])
            ident = pers.tile([128, 128], BF16, tag="ident", name="ident")
            nc.sync.dma_start(ident[:], ident_d[:])
            qkc = pers.tile([128, 6], F32, tag="qkc", name="qkc")
            nc.sync.dma_start(qkc[:], qkc_d[:])
            ones_col = pers.tile([128, 1], BF16, tag="ones_col",
                                 name="ones_col")
            nc.vector.memset(ones_col[:], 1.0)

            # persistent results of phase B
            qTn = [pers.tile([128, S], BF16, tag=f"qTn{h}", name=f"qTn{h}")
                   for h in range(QH_PER_CORE)]
            kTr = pers.tile([128, S], BF16, tag="kTr", name="kTr")
            invk_cols = pers.tile([128, NT], F32, tag="invk_cols", name="invk_cols")
            vplus = [pers.tile([128, 129], BF16, tag=f"vplus{t}", name=f"vplus{t}")
                     for t in range(NT)]
            yT = [pers.tile([128, S], BF16, tag=f"yT{h}", name=f"yT{h}")
                  for h in range(QH_PER_CORE)]

            def proj_unit(w_tiles, col_lo, ncols):
                """x @ W.T in transposed layout: returns list of psum chunk
                tiles [128, 512] covering out[hd(col_lo..), s]."""
                chunks = []
                for c in range(4):
                    pch = ps_big.tile([128, 512], F32, tag="big", name="big")
                    for dblk in range(NT):
                        nc.tensor.matmul(
                            pch[:ncols, :],
                            w_tiles[dblk][:, col_lo:col_lo + ncols],
                            xt[dblk][:, 512 * c:512 * (c + 1)],
                            start=(dblk == 0),
                            stop=(dblk == NT - 1),
                        )
                    chunks.append(pch)
                return chunks

            def psum_to_sbuf(chunks, dtype, ncols=128):
                t = tp_q.tile([128, S], dtype, tag="traw", name="traw")
                for c, pch in enumerate(chunks):
                    nc.vector.tensor_copy(
                        t[:ncols, 512 * c:512 * (c + 1)], pch[:ncols, :])
                return t

            def rope(src, dst):
                """dst = rotate(src): dst = src*cosF + halfswap(src)*sinF.

                cosF = [cos; cos], sinF = [sin; -sin]; the half-swap (the only
                cross-partition move) goes through SBUF->SBUF DMA since DVE
                lanes are partition-locked."""
                sw = tp_q.tile([128, S], BF16, tag="qsw", name="qsw")
                nc.sync.dma_start(sw[0:64, :], src[64:128, :])
                nc.sync.dma_start(sw[64:128, :], src[0:64, :])
                t1 = tp_half.tile([128, S], F32, tag="rt1", name="rt1")
                t2 = tp_half.tile([128, S], F32, tag="rt2", name="rt2")
                nc.vector.tensor_mul(t1[:], src[:], cost[:])
                nc.vector.tensor_mul(t2[:], sw[:], sint[:])
                nc.vector.tensor_add(dst[:], t1[:], t2[:])

            def inv_rms_row(src, scale_ap, bias_ap, rowname):
                """1/sqrt(colsum(src^2)*scale + bias) as a DRAM row [1, S].

                Column sums over the 128 partitions go through the PE (ones
                lhsT); sqrt/recip run on the [1, S] row."""
                sq = tp_q.tile([128, S], BF16, tag="qsw", name="sq")
                nc.vector.tensor_mul(sq[:], src[:], src[:])
                rrow = tp_row.tile([1, S], F32, tag="rrow", name="rrow")
                for c in range(4):
                    rp = ps_sm.tile([128, 512], F32, tag="sm", name="rowps")
                    nc.tensor.matmul(rp[0:1, :], ones_col[:],
                                     sq[:, 512 * c:512 * (c + 1)],
                                     start=True, stop=True)
                    nc.scalar.activation(rrow[0:1, 512 * c:512 * (c + 1)],
                                         rp[0:1, :], AF.Sqrt,
                                         scale=scale_ap, bias=bias_ap)
                nc.vector.reciprocal(rrow[:], rrow[:])
                rowdr = dram.tile([1, S], F32, tag="rowdr", name=rowname,
                                  bufs=2)
                nc.sync.dma_start(rowdr[:], rrow[:])
                return rowdr

            # ---- Phase B: projections + norms + rope ----
            for h in range(QH_PER_CORE):
                chunks = proj_unit(wqt, 128 * h, 128)
                qraw = psum_to_sbuf(chunks, BF16)
                qTr = tp_q.tile([128, S], BF16, tag="tr2", name="tr2")
                rope(qraw, qTr)
                rowdr = inv_rms_row(qTr, qkc[0:1, 2 * h:2 * h + 1],
                                    qkc[0:1, 2 * h + 1:2 * h + 2],
                                    f"invq_row{h}")
                inv_bc = tp_f32.tile([128, S], F32, tag="nchain",
                                     name="inv_bc")
                nc.sync.dma_start(inv_bc[:],
                                  rowdr[0:1, :].broadcast_to([128, S]))
                nc.vector.tensor_mul(qTn[h][:], qTr[:], inv_bc[:])

            # k unit
            chunks = proj_unit(wkt, 0, 128)
            kraw = psum_to_sbuf(chunks, BF16)
            rope(kraw, kTr)
            rowdr = inv_rms_row(kTr, qkc[0:1, 4:5], qkc[0:1, 5:6],
                                "invk_row")
            nc.sync.dma_start(
                invk_cols[:],
                rowdr[0:1, :].rearrange("one (j p) -> p (one j)", p=128))

            # v unit
            chunks = proj_unit(wvt, 0, 128)
            vT = psum_to_sbuf(chunks, BF16)
            for t in range(NT):
                trp = ps_sm.tile([128, 128], BF16, tag="sm", name="sm")
                nc.tensor.transpose(trp[:], vT[:, 128 * t:128 * (t + 1)], ident[:])
                nc.vector.tensor_copy(vplus[t][:, 0:128], trp[:])
                nc.vector.memset(vplus[t][:, 128:129], 1.0)

            # ---- Phase C: attention per head, q-tile batches of 8 ----
            def proj_out(i):
                """output projection for s-tile i (needs yT of both heads)."""
                for c in range(4):
                    pp = ps_big.tile([128, 512], F32, tag="big", name="big")
                    nc.tensor.matmul(pp[:], yT[0][:, 128 * i:128 * (i + 1)],
                                     pt[0][:, 512 * c:512 * (c + 1)],
                                     start=True, stop=False)
                    nc.tensor.matmul(pp[:], yT[1][:, 128 * i:128 * (i + 1)],
                                     pt[1][:, 512 * c:512 * (c + 1)],
                                     start=False, stop=True)
                    ob = tp_out.tile([128, 512], F32, tag="ob", name="ob")
                    if c % 2 == 0:
                        nc.vector.tensor_copy(ob[:], pp[:])
                    else:
                        nc.scalar.copy(ob[:], pp[:])
                    nc.sync.dma_start(
                        out_d[128 * i:128 * (i + 1), 512 * c:512 * (c + 1)],
                        ob[:])

            for h in range(QH_PER_CORE):
                for (lo, hi) in ((0, 8), (8, 16)):
                    nb = (hi - lo + 2) // 3
                    banks = [ps_big.tile([128, 512], F32, tag="big", name="big")
                             for _ in range(nb)]
                    imax_of_bank = [min(lo + 3 * b + 2, hi - 1)
                                    for b in range(nb)]
                    for j in range(hi):
                        qlo = max(lo * 128, j * 128)
                        w = hi * 128 - qlo
                        probs = tp_probs.tile([128, 1024], BF16, tag="probs", name="probs")
                        nchunk = (w + 511) // 512
                        for c in range(nchunk):
                            cw = min(512, w - 512 * c)
                            sc = ps_sm.tile([128, 512], F32, tag="sm", name="sm")
                            nc.tensor.matmul(
                                sc[:, :cw],
                                kTr[:, 128 * j:128 * (j + 1)],
                                qTn[h][:, qlo + 512 * c: qlo + 512 * c + cw],
                                start=True, stop=True)
                            nc.scalar.activation(
                                probs[:, 512 * c:512 * c + cw], sc[:, :cw],
                                AF.Exp, scale=invk_cols[:, j:j + 1])
                        if j >= lo:
                            nc.vector.tensor_mul(
                                probs[:, 0:128], probs[:, 0:128], maskd[:])
                        for i in range(max(lo, j), hi):
                            b, sub = divmod(i - lo, 3)
                            off = 128 * i - qlo
                            nc.tensor.matmul(
                                banks[b][:, 129 * sub:129 * sub + 129],
                                probs[:, off:off + 128],
                                vplus[j][:],
                                start=(j == 0 and sub == 0),
                                stop=(j == i and i == imax_of_bank[b]))
                        # extract completed bank (all its accums final)
                        for b in range(nb):
                            if j == imax_of_bank[b] and j >= lo:
                                for i in range(lo + 3 * b,
                                               min(lo + 3 * b + 3, hi)):
                                    sub = (i - lo) % 3
                                    rec = tp_rec.tile([128, 1], F32, tag="rec", name="rec")
                                    nc.vector.reciprocal(
                                        rec[:],
                                        banks[b][:, 129 * sub + 128:
                                                 129 * sub + 129])
                                    y = tp_y.tile([128, 128], BF16, tag="y", name="y")
                                    nc.vector.tensor_scalar_mul(
                                        y[:],
                                        banks[b][:, 129 * sub:129 * sub + 128],
                                        rec[:])
                                    trp = ps_sm.tile([128, 128], BF16, tag="sm", name="sm")
                                    nc.tensor.transpose(trp[:], y[:], ident[:])
                                    nc.vector.tensor_copy(
                                        yT[h][:, 128 * i:128 * (i + 1)], trp[:])
                                    if h == 1:
                                        proj_out(i)
    return nc


_NC_CACHE = None


def _get_nc():
    global _NC_CACHE
    if _NC_CACHE is None:
        _NC_CACHE = _build_program()
    return _NC_CACHE


def _host_prep(x, Wq, Wk, Wv, Wproj, q_gain):
    """Build the 8 per-core input maps (numpy, host side)."""
    x2 = np.asarray(x, np.float32).reshape(S, D)
    xt = np.ascontiguousarray(x2.T).astype(NP_BF16)

    inv_freq = 1.0 / (BASE ** (np.arange(0, HD, 2, dtype=np.float32) / HD))
    t = np.arange(S, dtype=np.float32)
    freqs = np.outer(t, inv_freq)                     # [S, 64]
    cos_h = np.cos(freqs).T                           # [64, S]
    sin_h = np.sin(freqs).T
    cosT = np.ascontiguousarray(
        np.concatenate([cos_h, cos_h], axis=0)).astype(NP_BF16)
    sinT = np.ascontiguousarray(
        np.concatenate([sin_h, -sin_h], axis=0)).astype(NP_BF16)

    maskd = np.triu(np.ones((128, 128), np.float32)).astype(NP_BF16)
    ident = np.eye(128, dtype=np.float32).astype(NP_BF16)

    Wq = np.asarray(Wq, np.float32)
    Wk = np.asarray(Wk, np.float32)
    Wv = np.asarray(Wv, np.float32)
    Wproj = np.asarray(Wproj, np.float32)
    q_gain = np.asarray(q_gain, np.float32)

    in_maps = []
    for core in range(NCORES):
        kv = core // 2
        wqt = np.ascontiguousarray(
            Wq[M_PER_CORE * core:M_PER_CORE * (core + 1), :].T).astype(NP_BF16)
        wkt = np.ascontiguousarray(
            Wk[HD * kv:HD * (kv + 1), :].T).astype(NP_BF16)
        wvt = np.ascontiguousarray(
            Wv[HD * kv:HD * (kv + 1), :].T).astype(NP_BF16)
        ptc = np.ascontiguousarray(
            Wproj[:, M_PER_CORE * core:M_PER_CORE * (core + 1)].T
        ).astype(NP_BF16)
        qkc = np.zeros((128, 6), np.float32)
        qkc[:, 4] = 1.0 / HD
        qkc[:, 5] = EPS
        for h in range(QH_PER_CORE):
            gain = float(q_gain[QH_PER_CORE * core + h])
            c = gain / math.sqrt(HD)
            if abs(c) < 1e-8:
                c = 1e-8
            qkc[:, 2 * h] = 1.0 / (HD * c * c)
            qkc[:, 2 * h + 1] = EPS / (c * c)
        in_maps.append({
            "xt": xt,
            "wqt": wqt,
            "wkt": wkt,
            "wvt": wvt,
            "pt": ptc,
            "cost": cosT,
            "sint": sinT,
            "maskd": maskd,
            "ident": ident,
            "qkc": qkc,
        })
    return in_maps


def kernel(**inputs):
    x = inputs["x"]
    in_maps = _host_prep(x, inputs["Wq"], inputs["Wk"], inputs["Wv"],
                         inputs["Wproj"], inputs["q_gain"])
    nc = _get_nc()
    res = run_bass_kernel_spmd(nc, in_maps, list(range(NCORES)))
    parts = [np.asarray(res.results[i]["partial"]) for i in range(NCORES)]
    out = np.sum(np.stack(parts, 0), axis=0, dtype=np.float64)
    return out.astype(np.float32).reshape(1, S, D)
